# revision 1
# baseline (speedup 1.0000x reference)
"""Bidirectional Mamba TRN2 kernel (8 NeuronCores, SPMD).

Sharding: 24 (b, dtile) pairs of 128 d_inner-channels; core c owns b = c//4 and
dtiles (c%4)*3 + {0,1,2}, BOTH scan directions. The x_proj contraction over the
full d_inner is completed with a per-batch AllReduce (groups {0..3}, {4..7}) of
partial `dbl` tensors. out_proj partials are summed on the host.

Per-direction scan runs as native DVE tensor_tensor_scan along L (d channels on
partitions, one scan per state index n), backward direction via negative-stride
APs. L is processed in 4 quarters with scan-state chaining; the bidirectional
output accumulator m is spilled to DRAM between the creating and completing
quarter iterations.
"""
import os
import numpy as np
from contextlib import ExitStack

import concourse.bass as bass
import concourse.bacc as bacc
import concourse.tile as tile
from concourse import mybir, library_config
from concourse.bass_utils import run_bass_kernel_spmd

B, L, D = 2, 4096, 768
DI, DS, DTR, KC = 1536, 16, 48, 4
NCORES = 8
NPAIR = 3                 # dtiles per core
P = 128
NKT = D // P              # 6 K-tiles for in_proj
LC = 512                  # matmul free-dim chunk
NLC = L // LC             # 8
NQ = 4                    # L quarters
LQ = L // NQ              # 1024
NLCQ = LQ // LC           # 2
E = DTR + 2 * DS          # 80

f32 = mybir.dt.float32
f32r = mybir.dt.float32r
bf16 = mybir.dt.bfloat16
ALU = mybir.AluOpType
AF = mybir.ActivationFunctionType


def build_module(stop_after='full'):
    nc = bacc.Bacc("TRN2", target_bir_lowering=False, debug=False,
                   num_devices=NCORES)

    # ---- external inputs (per-core data; same tensor names on all cores) ----
    hT = nc.dram_tensor("hT", [D, L], f32, kind="ExternalInput")
    w_in = nc.dram_tensor("w_in", [D, 2 * NPAIR * P], f32, kind="ExternalInput")
    convw = nc.dram_tensor("convw", [2, NPAIR, P, KC], f32, kind="ExternalInput")
    convb = nc.dram_tensor("convb", [2, NPAIR, P], f32, kind="ExternalInput")
    w_xp = nc.dram_tensor("w_xp", [2, NPAIR * P, E], f32, kind="ExternalInput")
    w_dt = nc.dram_tensor("w_dt", [2, NPAIR, DTR, P], f32, kind="ExternalInput")
    dt_bias = nc.dram_tensor("dt_bias", [2, NPAIR, P], f32, kind="ExternalInput")
    Acol = nc.dram_tensor("Acol", [2, NPAIR, P, DS], f32, kind="ExternalInput")
    Dvec = nc.dram_tensor("Dvec", [2, NPAIR, P], f32, kind="ExternalInput")
    w_out = nc.dram_tensor("w_out", [NPAIR, P, D], f32, kind="ExternalInput")
    out_part = nc.dram_tensor("out_part", [D, L], f32, kind="ExternalOutput")

    # ---- internal DRAM ----
    u16_dram = nc.dram_tensor("u16_dram", [2, NPAIR, P, L], bf16)
    z16_dram = nc.dram_tensor("z16_dram", [NPAIR, P, L], bf16)
    m_spill = nc.dram_tensor("m_spill", [NQ, NPAIR, P, LQ], f32)
    cc_in = nc.dram_tensor("cc_in", [2, E, L], f32)
    cc_out = nc.dram_tensor("cc_out", [2, E, L], f32)

    with tile.TileContext(nc) as tc, ExitStack() as top:
        wp = top.enter_context(tc.tile_pool(name="weights", bufs=1))
        pp = top.enter_context(tc.tile_pool(name="persist", bufs=1))

        nc.gpsimd.load_library(library_config.proxy)

        # ---- weight staging (P2-lifetime weights only; P1 weights below) ----
        w_dt_sb = wp.tile([DTR, 2, NPAIR, P], f32, tag="w_dt", name="w_dt")
        nc.sync.dma_start(w_dt_sb[:], w_dt.ap().rearrange("d j r p -> r d j p"))
        dtb_sb = wp.tile([P, 2, NPAIR], f32, tag="dtb", name="dtb")
        nc.sync.dma_start(dtb_sb[:], dt_bias.ap().rearrange("d j p -> p d j"))
        Acol_sb = wp.tile([P, 2, NPAIR, DS], f32, tag="Acol", name="Acol")
        nc.sync.dma_start(Acol_sb[:], Acol.ap().rearrange("d j p n -> p d j n"))
        D_sb = wp.tile([P, 2, NPAIR], f32, tag="Dsb", name="Dsb")
        nc.sync.dma_start(D_sb[:], Dvec.ap().rearrange("d j p -> p d j"))
        w_out_sb = wp.tile([P, NPAIR, D], f32, tag="w_out", name="w_out")
        nc.sync.dma_start(w_out_sb[:], w_out.ap().rearrange("j p c -> p j c"))

        # =========== Phase 1: in_proj, z-silu, conv, u, dbl partials ==========
        with ExitStack() as p1:
            wp1 = p1.enter_context(tc.tile_pool(name="p1w", bufs=1))
            x16p = p1.enter_context(tc.tile_pool(name="x16", bufs=1))
            tp1 = p1.enter_context(tc.tile_pool(name="p1tmp", bufs=2))
            bp1 = p1.enter_context(tc.tile_pool(name="p1big", bufs=1))
            up1 = p1.enter_context(tc.tile_pool(name="p1u", bufs=1))
            dblp = p1.enter_context(tc.tile_pool(name="dblsb", bufs=1))

            w_in_sb = wp1.tile([P, NKT, 2 * NPAIR * P], f32, tag="w_in", name="w_in")
            nc.sync.dma_start(w_in_sb[:],
                              w_in.ap().rearrange("(kt p) c -> p kt c", p=P))
            convw_sb = wp1.tile([P, 2, NPAIR, KC], f32, tag="convw", name="convw")
            nc.sync.dma_start(convw_sb[:],
                              convw.ap().rearrange("d j p k -> p d j k"))
            convb_sb = wp1.tile([P, 2, NPAIR], f32, tag="convb", name="convb")
            nc.sync.dma_start(convb_sb[:],
                              convb.ap().rearrange("d j p -> p d j"))
            w_xp_sb = wp1.tile([P, 2, NPAIR, E], f32, tag="w_xp", name="w_xp")
            nc.sync.dma_start(w_xp_sb[:],
                              w_xp.ap().rearrange("d (j p) e -> p d j e", p=P))

            x16 = [x16p.tile([P, L], bf16, tag=f"x16_{j}", name=f"x16_{j}") for j in range(NPAIR)]
            dbl_sb = [dblp.tile([E, L], f32, tag=f"dbl_{d}", name=f"dbl_{d}") for d in range(2)]

            with ExitStack() as s1:
                xzps = s1.enter_context(
                    tc.tile_pool(name="xzps", bufs=1, space="PSUM"))
                acc_ps = [[xzps.tile([P, LC], f32, tag=f"xz{j}{s}", name=f"xz{j}{s}")
                           for s in range(2)] for j in range(NPAIR)]
                for lc in range(NLC):
                    cols = slice(lc * LC, (lc + 1) * LC)
                    for kt in range(NKT):
                        rhs = tp1.tile([P, LC], f32, tag="rhs", name="rhs")
                        nc.sync.dma_start(rhs[:], hT.ap()[kt * P:(kt + 1) * P, cols])
                        for j in range(NPAIR):
                            for s in range(2):
                                wcol = (j * 2 + s) * P
                                nc.tensor.matmul(
                                    acc_ps[j][s][:],
                                    w_in_sb[:, kt, wcol:wcol + P],
                                    rhs[:],
                                    start=(kt == 0), stop=(kt == NKT - 1))
                    for j in range(NPAIR):
                        # x -> SBUF bf16 (ScalarE evac with cast)
                        nc.scalar.copy(x16[j][:, cols], acc_ps[j][0][:])
                        # z -> silu(z) in bf16 -> DRAM
                        sg = tp1.tile([P, LC], f32, tag="zsg", name="zsg")
                        nc.scalar.activation(sg[:], acc_ps[j][1][:], AF.Sigmoid)
                        z16 = tp1.tile([P, LC], bf16, tag="z16", name="z16")
                        nc.vector.tensor_tensor(z16[:], acc_ps[j][1][:], sg[:],
                                                op=ALU.mult)
                        nc.sync.dma_start(z16_dram.ap()[j, :, cols], z16[:])

            # conv + silu -> u; dbl partial matmuls
            with ExitStack() as s2:
                dblps = s2.enter_context(
                    tc.tile_pool(name="dblps", bufs=2, space="PSUM"))
                for j in range(NPAIR):
                    u32 = {}
                    for dr in range(2):
                        acc = bp1.tile([P, L], f32, tag="cacc", name="cacc")
                        w = lambda k: convw_sb[:, dr, j, k:k + 1]
                        if dr == 0:   # taps k read x[t-3+k]
                            nc.vector.tensor_scalar_mul(acc[:], x16[j][:], w(3))
                            for k in range(3):
                                sh = 3 - k
                                nc.vector.scalar_tensor_tensor(
                                    acc[:, sh:L], x16[j][:, 0:L - sh], w(k),
                                    acc[:, sh:L], op0=ALU.mult, op1=ALU.add)
                        else:         # host-reversed taps j read x[t+j]
                            nc.vector.tensor_scalar_mul(acc[:], x16[j][:], w(0))
                            for jj in range(1, 4):
                                nc.vector.scalar_tensor_tensor(
                                    acc[:, 0:L - jj], x16[j][:, jj:L], w(jj),
                                    acc[:, 0:L - jj], op0=ALU.mult, op1=ALU.add)
                        nc.vector.tensor_scalar_add(acc[:], acc[:],
                                                    convb_sb[:, dr, j:j + 1])
                        sg = bp1.tile([P, L], f32, tag="usg", name="usg")
                        nc.scalar.activation(sg[:], acc[:], AF.Sigmoid)
                        u = up1.tile([P, L], f32, tag=f"u32_{dr}", name=f"u32_{dr}")
                        nc.vector.tensor_tensor(u[:], acc[:], sg[:], op=ALU.mult)
                        u32[dr] = u
                        u16 = tp1.tile([P, L], bf16, tag="u16st", name="u16st")
                        nc.vector.tensor_copy(u16[:], u[:])
                        nc.sync.dma_start(u16_dram.ap()[dr, j, :, :], u16[:])
                    for dr in range(2):
                        for lc in range(NLC):
                            cols = slice(lc * LC, (lc + 1) * LC)
                            dps = dblps.tile([E, LC], f32, tag="dblps", name="dblps")
                            nc.tensor.matmul(
                                dps[:], w_xp_sb[:, dr, j, :],
                                u32[dr][:, cols],
                                start=True, stop=True)
                            if j == 0:
                                nc.scalar.copy(dbl_sb[dr][:, cols], dps[:])
                            else:
                                nc.vector.tensor_tensor(
                                    dbl_sb[dr][:, cols], dbl_sb[dr][:, cols],
                                    dps[:], op=ALU.add)

            for dr in range(2):
                nc.sync.dma_start(cc_in.ap()[dr], dbl_sb[dr][:])

        # =========== AllReduce of dbl partials (per-batch groups) ============
        if stop_after != 'p1':
            nc.gpsimd.collective_compute(
                "AllReduce", ALU.add,
                replica_groups=[[0, 1, 2, 3], [4, 5, 6, 7]],
                ins=[cc_in.ap()], outs=[cc_out.ap()])

        # =========== Phase 2: delta, scans, y accumulation, out_proj =========
        if stop_after not in ('p1', 'cc'):
          with ExitStack() as p2:
                rows = p2.enter_context(tc.tile_pool(name="rows", bufs=1))
                st = p2.enter_context(tc.tile_pool(name="state", bufs=1))
                dp2 = p2.enter_context(tc.tile_pool(name="p2d", bufs=1))
                tp2 = p2.enter_context(tc.tile_pool(name="p2tmp", bufs=2))
                hp2 = p2.enter_context(tc.tile_pool(name="p2h", bufs=2))
                mp2 = p2.enter_context(tc.tile_pool(name="p2m", bufs=2))
                psd = p2.enter_context(tc.tile_pool(name="dtps", bufs=2, space="PSUM"))
                pso = p2.enter_context(tc.tile_pool(name="outps", bufs=2, space="PSUM"))

                # B/C rows in bf16: one (2*DS, L) tile per dir (gpsimd DMA casts)
                rows16 = {}
                for dr in range(2):
                    rt = rows.tile([2 * DS, L], bf16, tag=f"rows16_{dr}",
                                   name=f"rows16_{dr}")
                    nc.gpsimd.dma_start(rt[:], cc_out.ap()[dr, DTR:E, :])
                    rows16[dr] = rt

                state = {(dr, n, j): st.tile([P, 1], bf16, tag=f"st{dr}_{n}_{j}", name=f"st{dr}_{n}_{j}")
                         for dr in range(2) for n in range(DS) for j in range(NPAIR)}
                m32 = {}   # (q, j) -> live tile

                nq_run = {'it1': 1, 'it2': 2, 'it3': 3}.get(stop_after, NQ)
                for it in range(nq_run):
                    combos = [(0, it), (1, NQ - 1 - it)]
                    delta32, du16, u16q = {}, {}, {}
                    for ci, (dr, q) in enumerate(combos):
                        qcols = slice(q * LQ, (q + 1) * LQ)
                        dtlow = tp2.tile([DTR, LQ], f32, tag="dtlow", name="dtlow")
                        nc.sync.dma_start(dtlow[:], cc_out.ap()[dr, 0:DTR, qcols])
                        for j in range(NPAIR):
                            dlt = dp2.tile([P, LQ], bf16, tag=f"delta{ci}_{j}", name=f"delta{ci}_{j}")
                            for lc in range(NLCQ):
                                c0, c1 = lc * LC, (lc + 1) * LC
                                dps = psd.tile([P, LC], f32, tag="dtps", name="dtps")
                                nc.tensor.matmul(
                                    dps[:], w_dt_sb[:, dr, j, :],
                                    dtlow[:, c0:c1],
                                    start=True, stop=True)
                                e32 = tp2.tile([P, LC], f32, tag="e32", name="e32")
                                nc.scalar.activation(e32[:], dps[:], AF.Exp,
                                                     bias=dtb_sb[:, dr, j:j + 1])
                                nc.scalar.activation(dlt[:, c0:c1], e32[:], AF.Ln,
                                                     bias=1.0)
                            delta32[ci, j] = dlt
                            uq = dp2.tile([P, LQ], bf16, tag=f"u16q{ci}_{j}", name=f"u16q{ci}_{j}")
                            nc.sync.dma_start(uq[:], u16_dram.ap()[dr, j, :, qcols])
                            u16q[ci, j] = uq
                            du = dp2.tile([P, LQ], bf16, tag=f"du{ci}_{j}", name=f"du{ci}_{j}")
                            nc.vector.tensor_tensor(du[:], dlt[:], uq[:], op=ALU.mult)
                            du16[ci, j] = du

                    for ci, (dr, q) in enumerate(combos):
                        creating = (q in (0, 1)) if dr == 0 else (q in (2, 3))
                        if not creating:
                            for j in range(NPAIR):
                                m = mp2.tile([P, LQ], f32, tag=f"m{j}", name=f"m{j}")
                                nc.sync.dma_start(m[:], m_spill.ap()[q, j])
                                m32[q, j] = m
                        for n in range(DS):
                            qcols = slice(q * LQ, (q + 1) * LQ)
                            bstage = hp2.tile([1, LQ], bf16, tag="bstage", name="bstage")
                            nc.sync.dma_start(bstage[:], rows16[dr][n:n + 1, qcols])
                            Brep = hp2.tile([P, LQ], bf16, tag="Brep", name="Brep")
                            nc.gpsimd.partition_broadcast(Brep[:], bstage[:])
                            cstage = hp2.tile([1, LQ], bf16, tag="cstage", name="cstage")
                            nc.sync.dma_start(cstage[:], rows16[dr][DS + n:DS + n + 1, qcols])
                            Crep = hp2.tile([P, LQ], bf16, tag="Crep", name="Crep")
                            nc.gpsimd.partition_broadcast(Crep[:], cstage[:])
                            for j in range(NPAIR):
                                dA = hp2.tile([P, LQ], bf16, tag="dA", name="dA")
                                nc.scalar.activation(dA[:], delta32[ci, j][:], AF.Exp,
                                                     scale=Acol_sb[:, dr, j, n:n + 1])
                                d1 = hp2.tile([P, LQ], bf16, tag="d1", name="d1")
                                nc.vector.tensor_tensor(d1[:], du16[ci, j][:], Brep[:],
                                                        op=ALU.mult)
                                h = hp2.tile([P, LQ], bf16, tag="h", name="h")
                                stt = state[dr, n, j]
                                if dr == 0:
                                    init = 0.0 if q == 0 else stt[:, 0:1]
                                    nc.vector.tensor_tensor_scan(
                                        h[:], dA[:], d1[:], init,
                                        op0=ALU.mult, op1=ALU.add)
                                    if q < NQ - 1:
                                        nc.vector.tensor_copy(stt[:], h[:, LQ - 1:LQ])
                                else:
                                    init = 0.0 if q == NQ - 1 else stt[:, 0:1]
                                    nc.vector.tensor_tensor_scan(
                                        h[:, ::-1], dA[:, ::-1], d1[:, ::-1], init,
                                        op0=ALU.mult, op1=ALU.add)
                                    if q > 0:
                                        nc.vector.tensor_copy(stt[:], h[:, 0:1])
                                if creating and n == 0:
                                    m = mp2.tile([P, LQ], f32, tag=f"m{j}", name=f"m{j}")
                                    nc.vector.tensor_tensor(m[:], h[:], Crep[:],
                                                            op=ALU.mult)
                                    m32[q, j] = m
                                else:
                                    hc = hp2.tile([P, LQ], bf16, tag="hc", name="hc")
                                    nc.vector.tensor_tensor(hc[:], h[:], Crep[:],
                                                            op=ALU.mult)
                                    m = m32[q, j]
                                    nc.vector.tensor_tensor(m[:], m[:], hc[:],
                                                            op=ALU.add)
                        if creating:
                            for j in range(NPAIR):
                                nc.sync.dma_start(m_spill.ap()[q, j], m32[q, j][:])
                        else:
                            # D-terms, gating, out_proj for completed quarter q
                            qcols = slice(q * LQ, (q + 1) * LQ)
                            other = 1 - dr
                            for j in range(NPAIR):
                                m = m32[q, j]
                                nc.vector.scalar_tensor_tensor(
                                    m[:], u16q[ci, j][:], D_sb[:, dr, j:j + 1], m[:],
                                    op0=ALU.mult, op1=ALU.add)
                                uo = tp2.tile([P, LQ], bf16, tag="uoth", name="uoth")
                                nc.sync.dma_start(uo[:], u16_dram.ap()[other, j, :, qcols])
                                nc.vector.scalar_tensor_tensor(
                                    m[:], uo[:], D_sb[:, other, j:j + 1], m[:],
                                    op0=ALU.mult, op1=ALU.add)
                                zt = tp2.tile([P, LQ], bf16, tag="zq", name="zq")
                                nc.sync.dma_start(zt[:], z16_dram.ap()[j, :, qcols])
                                yg = tp2.tile([P, LQ], f32, tag=f"yg{j}", name=f"yg{j}")
                                nc.vector.tensor_tensor(yg[:], m[:], zt[:], op=ALU.mult)
                                m32.pop((q, j))
                                if j == 0:
                                    ygs = {}
                                ygs[j] = yg
                            for ot in range(D // P):
                                for lc in range(NLCQ):
                                    c0 = q * LQ + lc * LC
                                    ops_ = pso.tile([P, LC], f32, tag="outps", name="outps")
                                    for j in range(NPAIR):
                                        nc.tensor.matmul(
                                            ops_[:],
                                            w_out_sb[:, j, ot * P:(ot + 1) * P],
                                            ygs[j][:, lc * LC:(lc + 1) * LC],
                                            start=(j == 0), stop=(j == NPAIR - 1))
                                    osb = tp2.tile([P, LC], f32, tag="osb", name="osb")
                                    nc.scalar.copy(osb[:], ops_[:])
                                    nc.sync.dma_start(
                                        out_part.ap()[ot * P:(ot + 1) * P, c0:c0 + LC],
                                        osb[:])
    nc.compile()
    return nc


def _prep_core_inputs(inputs, core):
    """Host-side slicing/transposition of full inputs for one core."""
    b = core // 4
    dtiles = [(core % 4) * NPAIR + k for k in range(NPAIR)]
    chans = np.concatenate([np.arange(dt * P, (dt + 1) * P) for dt in dtiles])

    hid = np.asarray(inputs['hidden_states'])
    w_in_full = np.asarray(inputs['in_proj_w'])
    w_out_full = np.asarray(inputs['out_proj_w'])

    per_dir = {}
    for d, sfx in enumerate(('a', 'b')):
        per_dir[d] = dict(
            cw=np.asarray(inputs[f'conv_w_{sfx}'])[chans],
            cb=np.asarray(inputs[f'conv_b_{sfx}'])[chans],
            xp=np.asarray(inputs[f'x_proj_{sfx}_w'])[:, chans],
            dtp=np.asarray(inputs[f'dt_proj_{sfx}_w'])[chans],
            dtb=np.asarray(inputs[f'dt_bias_{sfx}'])[chans],
            A=-np.exp(np.asarray(inputs[f'A_{sfx}_log'])[chans]),
            Dv=np.asarray(inputs[f'D_{sfx}'])[chans],
        )

    w_in_cols = np.empty((D, 2 * NPAIR * P), np.float32)
    for j in range(NPAIR):
        ch_j = chans[j * P:(j + 1) * P]
        w_in_cols[:, (2 * j) * P:(2 * j + 1) * P] = w_in_full[ch_j].T
        w_in_cols[:, (2 * j + 1) * P:(2 * j + 2) * P] = w_in_full[DI + ch_j].T

    convw = np.empty((2, NPAIR, P, KC), np.float32)
    for d in range(2):
        cw = per_dir[d]['cw'].reshape(NPAIR, P, KC)
        if d == 0:
            convw[d] = cw
        else:
            convw[d] = cw[:, :, ::-1]       # reversed taps for backward conv

    out = {
        'hT': np.ascontiguousarray(hid[b].T, dtype=np.float32),
        'w_in': np.ascontiguousarray(w_in_cols),
        'convw': np.ascontiguousarray(convw),
        'convb': np.ascontiguousarray(
            np.stack([per_dir[d]['cb'].reshape(NPAIR, P) for d in range(2)])),
        'w_xp': np.ascontiguousarray(
            np.stack([per_dir[d]['xp'].T for d in range(2)])),
        'w_dt': np.ascontiguousarray(
            np.stack([per_dir[d]['dtp'].reshape(NPAIR, P, DTR)
                      .transpose(0, 2, 1) for d in range(2)])),
        'dt_bias': np.ascontiguousarray(
            np.stack([per_dir[d]['dtb'].reshape(NPAIR, P) for d in range(2)])),
        'Acol': np.ascontiguousarray(
            np.stack([per_dir[d]['A'].reshape(NPAIR, P, DS) for d in range(2)])),
        'Dvec': np.ascontiguousarray(
            np.stack([per_dir[d]['Dv'].reshape(NPAIR, P) for d in range(2)])),
        'w_out': np.ascontiguousarray(
            w_out_full[:, chans].T.reshape(NPAIR, P, D)),
    }
    return {k: v.astype(np.float32) for k, v in out.items()}


_module_cache = {}


def _get_module():
    if 'nc' not in _module_cache:
        _module_cache['nc'] = build_module()
    return _module_cache['nc']


def kernel(**inputs):
    nc = _get_module()
    in_maps = [_prep_core_inputs(inputs, c) for c in range(NCORES)]
    res = run_bass_kernel_spmd(nc, in_maps, list(range(NCORES)))
    out = np.zeros((B, L, D), np.float32)
    for c in range(NCORES):
        out[c // 4] += res.results[c]['out_part'].T
    return out



# revision 5
# speedup vs baseline: 1.2194x; 1.2194x over previous
"""Bidirectional Mamba TRN2 kernel (8 NeuronCores, SPMD) — v2.

Sharding: core c owns batch c//4 and dtiles (c%4)*3+{0,1,2} (128 channels
each), both directions. x_proj partials AllReduced per batch group.

v2 layout decisions (from HW microbenchmarks):
- Backward direction runs in a time-reversed frame end-to-end: x is reversed
  once (x16r), conv/x_proj/scan/gating all operate forward on reversed data,
  and the host un-reverses the backward out_proj partial. All scans are
  forward (reversed-AP scans are 1.6x slower on DVE).
- The 16-state hC sum accumulates on the PE via identity-weight matmuls into
  PSUM (f32), not DVE adds.
- Conv taps are Act-engine scale-copies (per-channel tap weights) summed on
  PE with the conv bias folded into the SiLU activation bias; z/u SiLU fused
  on Act (silu table).
- d1/hc/du are bf16 DVE TTs (2x mode); dA is Act exp with per-partition A
  scale; B/C row broadcasts on GpSimd.
- in_proj matmul stays f32 (accuracy anchor); x_proj/dt_proj/out_proj bf16.
"""
import numpy as np
from contextlib import ExitStack

import ml_dtypes
import concourse.bass as bass
import concourse.bacc as bacc
import concourse.tile as tile
from concourse import mybir, library_config
from concourse.bass_utils import run_bass_kernel_spmd

B, L, D = 2, 4096, 768
DI, DS, DTR, KC = 1536, 16, 48, 4
NCORES = 8
NPAIR = 3                 # dtiles per core
P = 128
NKT = D // P              # 6 K-tiles for in_proj
LC = 512                  # matmul free-dim chunk
NLC = L // LC             # 8
NQ = 4                    # L quarters for phase 2
LQ = L // NQ              # 1024
NCQ = LQ // LC            # 2 chunks per quarter
E = DTR + 2 * DS          # 80

f32 = mybir.dt.float32
bf16 = mybir.dt.bfloat16
ALU = mybir.AluOpType
AF = mybir.ActivationFunctionType


def build_module():
    nc = bacc.Bacc("TRN2", target_bir_lowering=False, debug=False,
                   num_devices=NCORES)

    # ---- external inputs (per-core data; same tensor names on all cores) ----
    hT = nc.dram_tensor("hT", [D, L], f32, kind="ExternalInput")
    w_in = nc.dram_tensor("w_in", [D, 2 * NPAIR * P], f32, kind="ExternalInput")
    convw = nc.dram_tensor("convw", [2, NPAIR, P, KC], f32, kind="ExternalInput")
    convb = nc.dram_tensor("convb", [2, NPAIR, P], f32, kind="ExternalInput")
    w_xp = nc.dram_tensor("w_xp", [2, NPAIR * P, E], bf16, kind="ExternalInput")
    w_dt = nc.dram_tensor("w_dt", [2, NPAIR, DTR, P], bf16, kind="ExternalInput")
    dt_bias = nc.dram_tensor("dt_bias", [2, NPAIR, P], f32, kind="ExternalInput")
    Acol = nc.dram_tensor("Acol", [2, NPAIR, P, DS], f32, kind="ExternalInput")
    Dvec = nc.dram_tensor("Dvec", [2, NPAIR, P], f32, kind="ExternalInput")
    w_out = nc.dram_tensor("w_out", [NPAIR, P, D], bf16, kind="ExternalInput")
    ident_d = nc.dram_tensor("ident", [P, P], bf16, kind="ExternalInput")
    out_a = nc.dram_tensor("out_a", [D, L], f32, kind="ExternalOutput")
    out_b = nc.dram_tensor("out_b", [D, L], f32, kind="ExternalOutput")

    # ---- internal DRAM (collective buffers; half-major so each AllReduce
    # slice is contiguous) ----
    LH = L // 2
    cc_in = nc.dram_tensor("cc_in", [2, 2, E, LH], bf16)
    cc_out = nc.dram_tensor("cc_out", [2, 2, E, LH], bf16)

    with tile.TileContext(nc) as tc, ExitStack() as top:
        wp = top.enter_context(tc.tile_pool(name="weights", bufs=1))
        pp = top.enter_context(tc.tile_pool(name="persist", bufs=1))

        nc.gpsimd.load_library(library_config.proxy)

        # ---- persistent weights (w_in is P1-scoped, below) ----
        convw_sb = wp.tile([P, 2, NPAIR, KC], f32, tag="convw", name="convw")
        nc.sync.dma_start(convw_sb[:], convw.ap().rearrange("d j p k -> p d j k"))
        convb_sb = wp.tile([P, 2, NPAIR], f32, tag="convb", name="convb")
        nc.sync.dma_start(convb_sb[:], convb.ap().rearrange("d j p -> p d j"))
        w_xp_sb = wp.tile([P, 2, NPAIR, E], bf16, tag="w_xp", name="w_xp")
        nc.sync.dma_start(w_xp_sb[:],
                          w_xp.ap().rearrange("d (j p) e -> p d j e", p=P))
        w_dt_sb = wp.tile([DTR, 2, NPAIR, P], bf16, tag="w_dt", name="w_dt")
        nc.sync.dma_start(w_dt_sb[:], w_dt.ap().rearrange("d j r p -> r d j p"))
        dtb_sb = wp.tile([P, 2, NPAIR], f32, tag="dtb", name="dtb")
        nc.sync.dma_start(dtb_sb[:], dt_bias.ap().rearrange("d j p -> p d j"))
        Acol_sb = wp.tile([P, 2, NPAIR, DS], f32, tag="Acol", name="Acol")
        nc.sync.dma_start(Acol_sb[:], Acol.ap().rearrange("d j p n -> p d j n"))
        D_sb = wp.tile([P, 2, NPAIR], f32, tag="Dsb", name="Dsb")
        nc.sync.dma_start(D_sb[:], Dvec.ap().rearrange("d j p -> p d j"))
        w_out_sb = wp.tile([P, NPAIR, D], bf16, tag="w_out", name="w_out")
        nc.sync.dma_start(w_out_sb[:], w_out.ap().rearrange("j p c -> p j c"))
        ident = wp.tile([P, P], bf16, tag="ident", name="ident")
        nc.sync.dma_start(ident[:], ident_d.ap())

        # ---- persistent activations ----
        zs16 = [pp.tile([P, L], bf16, tag=f"zs{j}", name=f"zs{j}")
                for j in range(NPAIR)]
        zsr16 = [pp.tile([P, L], bf16, tag=f"zsr{j}", name=f"zsr{j}")
                 for j in range(NPAIR)]
        # u16[dir*NPAIR+j]; dir 1 stored in reversed frame
        u16 = [pp.tile([P, L], bf16, tag=f"u{i}", name=f"u{i}")
               for i in range(2 * NPAIR)]

        # =========== Phase 1: in_proj, silu(z), conv, u, x_proj ==========
        with ExitStack() as p1:
            xp_ = p1.enter_context(tc.tile_pool(name="p1x", bufs=1))
            wp1 = p1.enter_context(tc.tile_pool(name="p1w", bufs=1))
            rp1 = p1.enter_context(tc.tile_pool(name="p1rhs", bufs=3))
            tp1 = p1.enter_context(tc.tile_pool(name="p1tap", bufs=2))
            ep1 = p1.enter_context(tc.tile_pool(name="p1evac", bufs=2))

            w_in_sb = wp1.tile([P, NKT, 2 * NPAIR * P], f32, tag="w_in",
                               name="w_in")
            nc.sync.dma_start(w_in_sb[:],
                              w_in.ap().rearrange("(kt p) c -> p kt c", p=P))

            x16 = [xp_.tile([P, L], bf16, tag=f"x16_{j}", name=f"x16_{j}")
                   for j in range(NPAIR)]
            x16r = [xp_.tile([P, L], bf16, tag=f"x16r_{j}", name=f"x16r_{j}")
                    for j in range(NPAIR)]

            # --- in_proj (f32 matmuls) + evac x (copy) / z (silu) ---
            with ExitStack() as s1:
                xzps = s1.enter_context(
                    tc.tile_pool(name="xzps", bufs=1, space="PSUM"))
                acc_ps = [[xzps.tile([P, LC], f32, tag=f"xz{j}{s}",
                                     name=f"xz{j}{s}")
                           for s in range(2)] for j in range(NPAIR)]
                for lc in range(NLC):
                    cols = slice(lc * LC, (lc + 1) * LC)
                    for kt in range(NKT):
                        rhs = rp1.tile([P, LC], f32, tag="rhs", name="rhs")
                        nc.sync.dma_start(rhs[:],
                                          hT.ap()[kt * P:(kt + 1) * P, cols])
                        for j in range(NPAIR):
                            for s in range(2):
                                wcol = (j * 2 + s) * P
                                nc.tensor.matmul(
                                    acc_ps[j][s][:],
                                    w_in_sb[:, kt, wcol:wcol + P],
                                    rhs[:],
                                    start=(kt == 0), stop=(kt == NKT - 1))
                    for j in range(NPAIR):
                        nc.scalar.copy(x16[j][:, cols], acc_ps[j][0][:])
                        nc.scalar.activation(zs16[j][:, cols], acc_ps[j][1][:],
                                             AF.Silu)

            # --- reversed copies for the backward frame ---
            for j in range(NPAIR):
                nc.vector.tensor_copy(x16r[j][:], x16[j][:, ::-1])
                nc.vector.tensor_copy(zsr16[j][:], zs16[j][:, ::-1])

            # --- conv (Act scale-copy taps + PE accum) + silu -> u ---
            with ExitStack() as s2:
                cvps = s2.enter_context(
                    tc.tile_pool(name="cvps", bufs=2, space="PSUM"))
                for dr in range(2):
                    for j in range(NPAIR):
                        xs = x16[j] if dr == 0 else x16r[j]
                        ui = u16[dr * NPAIR + j]
                        for c in range(NLC):
                            c0, c1 = c * LC, (c + 1) * LC
                            taps = []
                            for k in range(KC):
                                tk = tp1.tile([P, LC], bf16, tag=f"tap{k}",
                                              name=f"tap{k}")
                                sh = (KC - 1) - k  # output offset for tap k
                                w_k = convw_sb[:, dr, j, k:k + 1]
                                if sh == 0 or c > 0:
                                    nc.scalar.activation(
                                        tk[:], xs[:, c0 - sh:c1 - sh],
                                        AF.Copy, scale=w_k)
                                else:
                                    nc.vector.memset(tk[:, 0:sh], 0.0)
                                    nc.scalar.activation(
                                        tk[:, sh:LC], xs[:, 0:LC - sh],
                                        AF.Copy, scale=w_k)
                                taps.append(tk)
                            cps = cvps.tile([P, LC], f32, tag="cps", name="cps")
                            for k in range(KC):
                                nc.tensor.matmul(cps[:], ident[:], taps[k][:],
                                                 start=(k == 0),
                                                 stop=(k == KC - 1))
                            # silu(conv + bias): bias folds into Act bias
                            nc.scalar.activation(ui[:, c0:c1], cps[:], AF.Silu,
                                                 bias=convb_sb[:, dr, j:j + 1])

            # --- x_proj: dbl partials (bf16 matmuls, PSUM-accum over j) ---
            with ExitStack() as s3:
                dbps = s3.enter_context(
                    tc.tile_pool(name="dbps", bufs=2, space="PSUM"))
                for dr in range(2):
                    for lc in range(NLC):
                        cols = slice(lc * LC, (lc + 1) * LC)
                        dps = dbps.tile([E, LC], f32, tag="dblps", name="dblps")
                        for j in range(NPAIR):
                            nc.tensor.matmul(dps[:], w_xp_sb[:, dr, j, :],
                                             u16[dr * NPAIR + j][:, cols],
                                             start=(j == 0),
                                             stop=(j == NPAIR - 1))
                        ev = ep1.tile([E, LC], bf16, tag="dblev", name="dblev")
                        nc.scalar.copy(ev[:], dps[:])
                        hh, hc0 = lc // (NLC // 2), (lc % (NLC // 2)) * LC
                        nc.sync.dma_start(
                            cc_in.ap()[dr, hh, :, hc0:hc0 + LC], ev[:])

        # =========== AllReduce of dbl partials (per (dir, half)) ============
        for dr in range(2):
            for h in range(2):
                nc.gpsimd.collective_compute(
                    "AllReduce", ALU.add,
                    replica_groups=[[0, 1, 2, 3], [4, 5, 6, 7]],
                    ins=[cc_in.ap()[dr, h]],
                    outs=[cc_out.ap()[dr, h]])

        # =========== Phase 2: delta, scans, hC accumulation, out_proj =========
        with ExitStack() as p2:
            stp = p2.enter_context(tc.tile_pool(name="state", bufs=1))
            dqp = p2.enter_context(tc.tile_pool(name="p2dt", bufs=2))
            djp = p2.enter_context(tc.tile_pool(name="p2dj", bufs=2))
            bcp = p2.enter_context(tc.tile_pool(name="p2bc", bufs=2))
            hp2 = p2.enter_context(tc.tile_pool(name="p2h", bufs=2))
            ygp = p2.enter_context(tc.tile_pool(name="p2yg", bufs=2))
            op2 = p2.enter_context(tc.tile_pool(name="p2o", bufs=3))
            psd = p2.enter_context(tc.tile_pool(name="dtps", bufs=1, space="PSUM"))
            psm = p2.enter_context(tc.tile_pool(name="mps", bufs=1, space="PSUM"))
            pso = p2.enter_context(tc.tile_pool(name="outps", bufs=1, space="PSUM"))

            state = {(dr, n, j): stp.tile([P, 1], bf16, tag=f"st{dr}_{n}_{j}",
                                          name=f"st{dr}_{n}_{j}")
                     for dr in range(2) for n in range(DS) for j in range(NPAIR)}

            for dr in range(2):
                out_dram = out_a if dr == 0 else out_b
                for q in range(NQ):
                    qsl = slice(q * LQ, (q + 1) * LQ)
                    qh, qo = q // 2, (q % 2) * LQ
                    dtlow = dqp.tile([DTR, LQ], bf16, tag="dtlow", name="dtlow")
                    nc.sync.dma_start(dtlow[:],
                                      cc_out.ap()[dr, qh, 0:DTR, qo:qo + LQ])


                    # δ, du, Du per j
                    du, Du = {}, {}
                    for j in range(NPAIR):
                        dlt = djp.tile([P, LQ], bf16, tag=f"dlt{j}",
                                       name=f"dlt{j}")
                        for c in range(NCQ):
                            c0, c1 = c * LC, (c + 1) * LC
                            dps = psd.tile([P, LC], f32, tag="dtps", name="dtps")
                            nc.tensor.matmul(dps[:], w_dt_sb[:, dr, j, :],
                                             dtlow[:, c0:c1],
                                             start=True, stop=True)
                            e32 = djp.tile([P, LC], f32, tag="e32", name="e32")
                            nc.scalar.activation(e32[:], dps[:], AF.Exp,
                                                 bias=dtb_sb[:, dr, j:j + 1])
                            nc.scalar.activation(dlt[:, c0:c1], e32[:], AF.Ln,
                                                 bias=1.0)
                        ut = u16[dr * NPAIR + j]
                        duj = djp.tile([P, LQ], bf16, tag=f"du{j}", name=f"du{j}")
                        nc.vector.tensor_tensor(duj[:], dlt[:], ut[:, qsl],
                                                op=ALU.mult)
                        du[j] = (dlt, duj)
                        Duj = djp.tile([P, LQ], bf16, tag=f"Du{j}", name=f"Du{j}")
                        nc.vector.tensor_scalar_mul(Duj[:], ut[:, qsl],
                                                    D_sb[:, dr, j:j + 1])
                        Du[j] = Duj

                    # hC accumulation in PSUM: m_ps[j][c] = sum_n h_n*C_n + Du
                    m_ps = [[psm.tile([P, LC], f32, tag=f"m{j}{c}",
                                      name=f"m{j}{c}") for c in range(NCQ)]
                            for j in range(NPAIR)]

                    for n in range(DS):
                        brow = bcp.tile([1, LQ], bf16, tag="brow", name="brow")
                        nc.sync.dma_start(
                            brow[:], cc_out.ap()[dr, qh, DTR + n, qo:qo + LQ])
                        Brep = bcp.tile([P, LQ], bf16, tag="Brep", name="Brep")
                        nc.gpsimd.partition_broadcast(Brep[:], brow[:])
                        crow = bcp.tile([1, LQ], bf16, tag="crow", name="crow")
                        nc.sync.dma_start(
                            crow[:],
                            cc_out.ap()[dr, qh, DTR + DS + n, qo:qo + LQ])
                        Crep = bcp.tile([P, LQ], bf16, tag="Crep", name="Crep")
                        nc.gpsimd.partition_broadcast(Crep[:], crow[:])
                        for j in range(NPAIR):
                            dlt, duj = du[j]
                            dA = hp2.tile([P, LQ], bf16, tag="dA", name="dA")
                            nc.scalar.activation(dA[:], dlt[:], AF.Exp,
                                                 scale=Acol_sb[:, dr, j, n:n + 1])
                            d1 = hp2.tile([P, LQ], bf16, tag="d1", name="d1")
                            nc.vector.tensor_tensor(d1[:], duj[:], Brep[:],
                                                    op=ALU.mult)
                            h = hp2.tile([P, LQ], bf16, tag="h", name="h")
                            stt = state[dr, n, j]
                            init = 0.0 if q == 0 else stt[:, 0:1]
                            nc.vector.tensor_tensor_scan(h[:], dA[:], d1[:],
                                                         init, op0=ALU.mult,
                                                         op1=ALU.add)
                            if q < NQ - 1:
                                nc.vector.tensor_copy(stt[:], h[:, LQ - 1:LQ])
                            hc = hp2.tile([P, LQ], bf16, tag="hc", name="hc")
                            nc.vector.tensor_tensor(hc[:], h[:], Crep[:],
                                                    op=ALU.mult)
                            for c in range(NCQ):
                                c0, c1 = c * LC, (c + 1) * LC
                                nc.tensor.matmul(m_ps[j][c][:], ident[:],
                                                 hc[:, c0:c1],
                                                 start=(n == 0), stop=False)
                    ygs = []
                    for j in range(NPAIR):
                        zt = zs16[j] if dr == 0 else zsr16[j]
                        yg = ygp.tile([P, LQ], bf16, tag=f"yg{j}", name=f"yg{j}")
                        for c in range(NCQ):
                            c0, c1 = c * LC, (c + 1) * LC
                            nc.tensor.matmul(m_ps[j][c][:], ident[:],
                                             Du[j][:, c0:c1],
                                             start=False, stop=True)
                            nc.vector.tensor_tensor(
                                yg[:, c0:c1], m_ps[j][c][:],
                                zt[:, q * LQ + c0:q * LQ + c1], op=ALU.mult)
                        ygs.append(yg)

                    for ot in range(NKT):
                        for c in range(NCQ):
                            c0 = q * LQ + c * LC
                            ops_ = pso.tile([P, LC], f32, tag="outps",
                                            name="outps")
                            for j in range(NPAIR):
                                nc.tensor.matmul(
                                    ops_[:],
                                    w_out_sb[:, j, ot * P:(ot + 1) * P],
                                    ygs[j][:, c * LC:(c + 1) * LC],
                                    start=(j == 0), stop=(j == NPAIR - 1))
                            osb = op2.tile([P, LC], f32, tag="osb", name="osb")
                            nc.scalar.copy(osb[:], ops_[:])
                            nc.sync.dma_start(
                                out_dram.ap()[ot * P:(ot + 1) * P, c0:c0 + LC],
                                osb[:])
    nc.compile()
    return nc


def _prep_core_inputs(inputs, core):
    """Host-side slicing/transposition of full inputs for one core."""
    bf = ml_dtypes.bfloat16
    b = core // 4
    dtiles = [(core % 4) * NPAIR + k for k in range(NPAIR)]
    chans = np.concatenate([np.arange(dt * P, (dt + 1) * P) for dt in dtiles])

    hid = np.asarray(inputs['hidden_states'])
    w_in_full = np.asarray(inputs['in_proj_w'])
    w_out_full = np.asarray(inputs['out_proj_w'])

    per_dir = {}
    for d, sfx in enumerate(('a', 'b')):
        per_dir[d] = dict(
            cw=np.asarray(inputs[f'conv_w_{sfx}'])[chans],
            cb=np.asarray(inputs[f'conv_b_{sfx}'])[chans],
            xp=np.asarray(inputs[f'x_proj_{sfx}_w'])[:, chans],
            dtp=np.asarray(inputs[f'dt_proj_{sfx}_w'])[chans],
            dtb=np.asarray(inputs[f'dt_bias_{sfx}'])[chans],
            A=-np.exp(np.asarray(inputs[f'A_{sfx}_log'])[chans]),
            Dv=np.asarray(inputs[f'D_{sfx}'])[chans],
        )

    w_in_cols = np.empty((D, 2 * NPAIR * P), np.float32)
    for j in range(NPAIR):
        ch_j = chans[j * P:(j + 1) * P]
        w_in_cols[:, (2 * j) * P:(2 * j + 1) * P] = w_in_full[ch_j].T
        w_in_cols[:, (2 * j + 1) * P:(2 * j + 2) * P] = w_in_full[DI + ch_j].T

    out = {
        'hT': np.ascontiguousarray(hid[b].T).astype(np.float32),
        'w_in': np.ascontiguousarray(w_in_cols).astype(np.float32),
        'convw': np.ascontiguousarray(
            np.stack([per_dir[d]['cw'].reshape(NPAIR, P, KC)
                      for d in range(2)])).astype(np.float32),
        'convb': np.ascontiguousarray(
            np.stack([per_dir[d]['cb'].reshape(NPAIR, P)
                      for d in range(2)])).astype(np.float32),
        'w_xp': np.ascontiguousarray(
            np.stack([per_dir[d]['xp'].T for d in range(2)])).astype(bf),
        'w_dt': np.ascontiguousarray(
            np.stack([per_dir[d]['dtp'].reshape(NPAIR, P, DTR)
                      .transpose(0, 2, 1) for d in range(2)])).astype(bf),
        'dt_bias': np.ascontiguousarray(
            np.stack([per_dir[d]['dtb'].reshape(NPAIR, P)
                      for d in range(2)])).astype(np.float32),
        'Acol': np.ascontiguousarray(
            np.stack([per_dir[d]['A'].reshape(NPAIR, P, DS)
                      for d in range(2)])).astype(np.float32),
        'Dvec': np.ascontiguousarray(
            np.stack([per_dir[d]['Dv'].reshape(NPAIR, P)
                      for d in range(2)])).astype(np.float32),
        'w_out': np.ascontiguousarray(
            w_out_full[:, chans].T.reshape(NPAIR, P, D)).astype(bf),
        'ident': np.eye(P, dtype=np.float32).astype(bf),
    }
    return out


_module_cache = {}


def _get_module():
    if 'nc' not in _module_cache:
        _module_cache['nc'] = build_module()
    return _module_cache['nc']


def kernel(**inputs):
    nc = _get_module()
    in_maps = [_prep_core_inputs(inputs, c) for c in range(NCORES)]
    res = run_bass_kernel_spmd(nc, in_maps, list(range(NCORES)))
    out = np.zeros((B, L, D), np.float32)
    for c in range(NCORES):
        oa = np.asarray(res.results[c]['out_a'], np.float32)
        ob = np.asarray(res.results[c]['out_b'], np.float32)
        out[c // 4] += oa.T + ob[:, ::-1].T
    return out


# revision 8
# speedup vs baseline: 1.3931x; 1.1425x over previous
"""Bidirectional Mamba TRN2 kernel (8 NeuronCores, SPMD) — v3.

Sharding: core c owns batch c//4 and dtiles (c%4)*3+{0,1,2} (128 channels
each), both directions. x_proj partials AllReduced per batch group
(groups {0..3}, {4..7}), split per (direction, L-half) so collectives
overlap compute.

Pipeline: phase 1 is j-major (in_proj j -> conv both dirs j -> u j), so the
first AllReduce fires ~200us in and phase 2 (DVE-bound scans) overlaps the
rest of phase 1.

Engine assignment (from HW microbenchmarks):
- All scans run forward; the backward direction lives in a time-reversed
  frame (x reversed once on DVE; host un-reverses the backward out partial).
- 16-state hC sums accumulate on PE via identity matmuls into PSUM (f32).
- Conv taps: Act scale-copies summed on PE, bias folded into SiLU bias.
- d1/hc/du: bf16 DVE TTs (2x mode). dA: Act exp, per-partition A scale.
- B/C broadcasts: GpSimd partition_broadcast.
- All matmuls bf16 (f32 PSUM).
- delta softplus: batched Exp then batched Ln per quarter (avoids act-table
  thrash between exp/ln function sets).
"""
import numpy as np
from contextlib import ExitStack

import ml_dtypes
import concourse.bass as bass
import concourse.bacc as bacc
import concourse.tile as tile
from concourse import mybir, library_config
from concourse.bass_utils import run_bass_kernel_spmd

B, L, D = 2, 4096, 768
DI, DS, DTR, KC = 1536, 16, 48, 4
NCORES = 8
NPAIR = 3                 # dtiles per core
P = 128
NKT = D // P              # 6 K-tiles for in_proj
LC = 512                  # matmul free-dim chunk
NLC = L // LC             # 8
NQ = 4                    # L quarters for phase 2
LQ = L // NQ              # 1024
NCQ = LQ // LC            # 2 chunks per quarter
E = DTR + 2 * DS          # 80
LH = L // 2

f32 = mybir.dt.float32
bf16 = mybir.dt.bfloat16
ALU = mybir.AluOpType
AF = mybir.ActivationFunctionType


def build_module():
    nc = bacc.Bacc("TRN2", target_bir_lowering=False, debug=False,
                   num_devices=NCORES)

    # ---- external inputs ----
    hT = nc.dram_tensor("hT", [D, L], bf16, kind="ExternalInput")
    w_in = nc.dram_tensor("w_in", [D, 2 * NPAIR * P], bf16, kind="ExternalInput")
    convw = nc.dram_tensor("convw", [2, NPAIR, P, KC], f32, kind="ExternalInput")
    convb = nc.dram_tensor("convb", [2, NPAIR, P], f32, kind="ExternalInput")
    w_xp = nc.dram_tensor("w_xp", [2, NPAIR * P, E], bf16, kind="ExternalInput")
    w_dt = nc.dram_tensor("w_dt", [2, NPAIR, DTR, P], bf16, kind="ExternalInput")
    dt_bias = nc.dram_tensor("dt_bias", [2, NPAIR, P], f32, kind="ExternalInput")
    Acol = nc.dram_tensor("Acol", [2, NPAIR, P, DS], f32, kind="ExternalInput")
    Dvec = nc.dram_tensor("Dvec", [2, NPAIR, P], f32, kind="ExternalInput")
    w_out = nc.dram_tensor("w_out", [NPAIR, P, D], bf16, kind="ExternalInput")
    ident_d = nc.dram_tensor("ident", [P, P], bf16, kind="ExternalInput")
    out_a = nc.dram_tensor("out_a", [D, L], f32, kind="ExternalOutput")
    out_b = nc.dram_tensor("out_b", [D, L], f32, kind="ExternalOutput")

    # ---- internal DRAM ----
    cc_in = nc.dram_tensor("cc_in", [2, 2, E, LH], bf16)   # (dir, half, E, LH)
    cc_out = nc.dram_tensor("cc_out", [2, 2, E, LH], bf16)
    u_dram = nc.dram_tensor("u_dram", [2, NPAIR, P, L], bf16)
    zs_dram = nc.dram_tensor("zs_dram", [NPAIR, P, L], bf16)
    zsr_dram = nc.dram_tensor("zsr_dram", [NPAIR, P, L], bf16)

    with tile.TileContext(nc) as tc, ExitStack() as top:
        wp = top.enter_context(tc.tile_pool(name="weights", bufs=1))
        # P2-hot pools first: low SBUF addresses, no aliasing with P1 pools
        stp = top.enter_context(tc.tile_pool(name="state", bufs=1))
        dqp = top.enter_context(tc.tile_pool(name="p2dt", bufs=2))
        djp = top.enter_context(tc.tile_pool(name="p2dj", bufs=1))
        e3p = top.enter_context(tc.tile_pool(name="p2e", bufs=1))
        bcp = top.enter_context(tc.tile_pool(name="p2bc", bufs=2))
        hp2 = top.enter_context(tc.tile_pool(name="p2h", bufs=2))
        ygp = top.enter_context(tc.tile_pool(name="p2yg", bufs=2))
        op2 = top.enter_context(tc.tile_pool(name="p2o", bufs=2))
        usp = top.enter_context(tc.tile_pool(name="p2us", bufs=2))
        zsp = top.enter_context(tc.tile_pool(name="p2zs", bufs=1))

        nc.gpsimd.load_library(library_config.proxy)

        # ---- persistent weights ----
        convw_sb = wp.tile([P, 2, NPAIR, KC], f32, tag="convw", name="convw")
        nc.sync.dma_start(convw_sb[:], convw.ap().rearrange("d j p k -> p d j k"))
        convb_sb = wp.tile([P, 2, NPAIR], f32, tag="convb", name="convb")
        nc.sync.dma_start(convb_sb[:], convb.ap().rearrange("d j p -> p d j"))
        w_xp_sb = wp.tile([P, 2, NPAIR, E], bf16, tag="w_xp", name="w_xp")
        nc.sync.dma_start(w_xp_sb[:],
                          w_xp.ap().rearrange("d (j p) e -> p d j e", p=P))
        w_dt_sb = wp.tile([DTR, 2, NPAIR, P], bf16, tag="w_dt", name="w_dt")
        nc.sync.dma_start(w_dt_sb[:], w_dt.ap().rearrange("d j r p -> r d j p"))
        dtb_sb = wp.tile([P, 2, NPAIR], f32, tag="dtb", name="dtb")
        nc.sync.dma_start(dtb_sb[:], dt_bias.ap().rearrange("d j p -> p d j"))
        Acol_sb = wp.tile([P, 2, NPAIR, DS], f32, tag="Acol", name="Acol")
        nc.sync.dma_start(Acol_sb[:], Acol.ap().rearrange("d j p n -> p d j n"))
        D_sb = wp.tile([P, 2, NPAIR], f32, tag="Dsb", name="Dsb")
        nc.sync.dma_start(D_sb[:], Dvec.ap().rearrange("d j p -> p d j"))
        w_out_sb = wp.tile([P, NPAIR, D], bf16, tag="w_out", name="w_out")
        nc.sync.dma_start(w_out_sb[:], w_out.ap().rearrange("j p c -> p j c"))
        ident = wp.tile([P, P], bf16, tag="ident", name="ident")
        nc.sync.dma_start(ident[:], ident_d.ap())

        # =========== Phase 1 (j-major) ==========
        with ExitStack() as p1:
            wp1 = p1.enter_context(tc.tile_pool(name="p1w", bufs=1))
            xp_ = p1.enter_context(tc.tile_pool(name="p1x", bufs=1))
            up_ = p1.enter_context(tc.tile_pool(name="p1u", bufs=1))
            rp1 = p1.enter_context(tc.tile_pool(name="p1rhs", bufs=3))
            tp1 = p1.enter_context(tc.tile_pool(name="p1tap", bufs=2))
            ep1 = p1.enter_context(tc.tile_pool(name="p1evac", bufs=2))
            psxz = p1.enter_context(tc.tile_pool(name="xzps", bufs=2, space="PSUM"))
            cvps = p1.enter_context(tc.tile_pool(name="cvps", bufs=2, space="PSUM"))
            dbps = p1.enter_context(tc.tile_pool(name="dbps", bufs=2, space="PSUM"))

            w_in_sb = wp1.tile([P, NKT, 2 * NPAIR * P], bf16, tag="w_in",
                               name="w_in")
            nc.sync.dma_start(w_in_sb[:],
                              w_in.ap().rearrange("(kt p) c -> p kt c", p=P))

            u16 = [up_.tile([P, L], bf16, tag=f"u{i}", name=f"u{i}")
                   for i in range(2 * NPAIR)]

            for j in range(NPAIR):
                x16 = xp_.tile([P, L], bf16, tag="x16", name=f"x16_{j}")
                x16r = xp_.tile([P, L], bf16, tag="x16r", name=f"x16r_{j}")
                zs = xp_.tile([P, L], bf16, tag="zs", name=f"zs_{j}")
                zsr = xp_.tile([P, L], bf16, tag="zsr", name=f"zsr_{j}")

                # --- in_proj for this j ---
                for lc in range(NLC):
                    cols = slice(lc * LC, (lc + 1) * LC)
                    psx = psxz.tile([P, LC], f32, tag="psx", name="psx")
                    psz = psxz.tile([P, LC], f32, tag="psz", name="psz")
                    for kt in range(NKT):
                        rhs = rp1.tile([P, LC], bf16, tag="rhs", name="rhs")
                        nc.sync.dma_start(rhs[:],
                                          hT.ap()[kt * P:(kt + 1) * P, cols])
                        nc.tensor.matmul(psx[:],
                                         w_in_sb[:, kt, (2 * j) * P:(2 * j + 1) * P],
                                         rhs[:], start=(kt == 0),
                                         stop=(kt == NKT - 1))
                        nc.tensor.matmul(psz[:],
                                         w_in_sb[:, kt,
                                                 (2 * j + 1) * P:(2 * j + 2) * P],
                                         rhs[:], start=(kt == 0),
                                         stop=(kt == NKT - 1))
                    nc.scalar.copy(x16[:, cols], psx[:])
                    nc.scalar.activation(zs[:, cols], psz[:], AF.Silu)

                # --- reversed frames ---
                nc.vector.tensor_copy(x16r[:], x16[:, ::-1])
                nc.vector.tensor_copy(zsr[:], zs[:, ::-1])
                nc.sync.dma_start(zs_dram.ap()[j], zs[:])
                nc.sync.dma_start(zsr_dram.ap()[j], zsr[:])

                # --- conv both dirs + silu -> u ---
                for dr in range(2):
                    xs = x16 if dr == 0 else x16r
                    ui = u16[dr * NPAIR + j]
                    for c in range(NLC):
                        c0, c1 = c * LC, (c + 1) * LC
                        taps = []
                        for k in range(KC):
                            tk = tp1.tile([P, LC], bf16, tag=f"tap{k}",
                                          name=f"tap{k}")
                            sh = (KC - 1) - k
                            w_k = convw_sb[:, dr, j, k:k + 1]
                            if sh == 0 or c > 0:
                                nc.scalar.activation(
                                    tk[:], xs[:, c0 - sh:c1 - sh],
                                    AF.Copy, scale=w_k)
                            else:
                                nc.vector.memset(tk[:, 0:sh], 0.0)
                                nc.scalar.activation(
                                    tk[:, sh:LC], xs[:, 0:LC - sh],
                                    AF.Copy, scale=w_k)
                            taps.append(tk)
                        cps = cvps.tile([P, LC], f32, tag="cps", name="cps")
                        for k in range(KC):
                            nc.tensor.matmul(cps[:], ident[:], taps[k][:],
                                             start=(k == 0), stop=(k == KC - 1))
                        nc.scalar.activation(ui[:, c0:c1], cps[:], AF.Silu,
                                             bias=convb_sb[:, dr, j:j + 1])
                    nc.sync.dma_start(u_dram.ap()[dr, j], ui[:])

            # --- x_proj partials + AllReduce per (dir, half) ---
            for dr in range(2):
                for h in range(2):
                    for lc2 in range(NLC // 2):
                        c0 = h * LH + lc2 * LC
                        cols = slice(c0, c0 + LC)
                        dps = dbps.tile([E, LC], f32, tag="dblps", name="dblps")
                        for j in range(NPAIR):
                            nc.tensor.matmul(dps[:], w_xp_sb[:, dr, j, :],
                                             u16[dr * NPAIR + j][:, cols],
                                             start=(j == 0),
                                             stop=(j == NPAIR - 1))
                        ev = ep1.tile([E, LC], bf16, tag="dblev", name="dblev")
                        nc.scalar.copy(ev[:], dps[:])
                        nc.sync.dma_start(
                            cc_in.ap()[dr, h, :, lc2 * LC:(lc2 + 1) * LC], ev[:])
                    nc.gpsimd.collective_compute(
                        "AllReduce", ALU.add,
                        replica_groups=[[0, 1, 2, 3], [4, 5, 6, 7]],
                        ins=[cc_in.ap()[dr, h]],
                        outs=[cc_out.ap()[dr, h]])

        # =========== Phase 2 ==========
        with ExitStack() as p2:
            psd = p2.enter_context(tc.tile_pool(name="dtps", bufs=1, space="PSUM"))
            psm = p2.enter_context(tc.tile_pool(name="mps", bufs=1, space="PSUM"))
            pso = p2.enter_context(tc.tile_pool(name="outps", bufs=1, space="PSUM"))

            state = {(dr, n, j): stp.tile([P, 1], bf16, tag=f"st{dr}_{n}_{j}",
                                          name=f"st{dr}_{n}_{j}")
                     for dr in range(2) for n in range(DS) for j in range(NPAIR)}

            for dr in range(2):
                out_dram = out_a if dr == 0 else out_b
                zdram = zs_dram if dr == 0 else zsr_dram
                for q in range(NQ):
                    qsl = slice(q * LQ, (q + 1) * LQ)
                    qh, qo = q // 2, (q % 2) * LQ
                    dtlow = dqp.tile([DTR, LQ], bf16, tag="dtlow", name="dtlow")
                    nc.sync.dma_start(dtlow[:],
                                      cc_out.ap()[dr, qh, 0:DTR, qo:qo + LQ])

                    # u / silu(z) streams for this quarter
                    ut, zt = {}, {}
                    for j in range(NPAIR):
                        u_t = usp.tile([P, LQ], bf16, tag=f"ut{j}", name=f"ut{j}")
                        nc.sync.dma_start(u_t[:], u_dram.ap()[dr, j, :, qsl])
                        ut[j] = u_t
                        z_t = zsp.tile([P, LQ], bf16, tag=f"zt{j}", name=f"zt{j}")
                        nc.sync.dma_start(z_t[:], zdram.ap()[j, :, qsl])
                        zt[j] = z_t

                    # delta: batched matmul+exp, then batched ln
                    e32 = {}
                    for j in range(NPAIR):
                        for c in range(NCQ):
                            c0, c1 = c * LC, (c + 1) * LC
                            dps = psd.tile([P, LC], f32, tag="dtps", name="dtps")
                            nc.tensor.matmul(dps[:], w_dt_sb[:, dr, j, :],
                                             dtlow[:, c0:c1],
                                             start=True, stop=True)
                            e = e3p.tile([P, LC], bf16, tag=f"e{j}{c}",
                                         name=f"e{j}{c}")
                            nc.scalar.activation(e[:], dps[:], AF.Exp,
                                                 bias=dtb_sb[:, dr, j:j + 1])
                            e32[j, c] = e
                    du, Du = {}, {}
                    for j in range(NPAIR):
                        dlt = djp.tile([P, LQ], bf16, tag=f"dlt{j}",
                                       name=f"dlt{j}")
                        for c in range(NCQ):
                            c0, c1 = c * LC, (c + 1) * LC
                            nc.scalar.activation(dlt[:, c0:c1], e32[j, c][:],
                                                 AF.Ln, bias=1.0)
                        duj = djp.tile([P, LQ], bf16, tag=f"du{j}", name=f"du{j}")
                        nc.vector.tensor_tensor(duj[:], dlt[:], ut[j][:],
                                                op=ALU.mult)
                        du[j] = (dlt, duj)
                        Duj = djp.tile([P, LQ], bf16, tag=f"Du{j}", name=f"Du{j}")
                        nc.vector.tensor_scalar_mul(Duj[:], ut[j][:],
                                                    D_sb[:, dr, j:j + 1])
                        Du[j] = Duj

                    # hC accumulation in PSUM
                    m_ps = [[psm.tile([P, LC], f32, tag=f"m{j}{c}",
                                      name=f"m{j}{c}") for c in range(NCQ)]
                            for j in range(NPAIR)]

                    for n in range(DS):
                        brow = bcp.tile([1, LQ], bf16, tag="brow", name="brow")
                        nc.sync.dma_start(
                            brow[:], cc_out.ap()[dr, qh, DTR + n, qo:qo + LQ])
                        Brep = bcp.tile([P, LQ], bf16, tag="Brep", name="Brep")
                        nc.gpsimd.partition_broadcast(Brep[:], brow[:])
                        crow = bcp.tile([1, LQ], bf16, tag="crow", name="crow")
                        nc.sync.dma_start(
                            crow[:],
                            cc_out.ap()[dr, qh, DTR + DS + n, qo:qo + LQ])
                        Crep = bcp.tile([P, LQ], bf16, tag="Crep", name="Crep")
                        nc.gpsimd.partition_broadcast(Crep[:], crow[:])
                        for j in range(NPAIR):
                            dlt, duj = du[j]
                            dA = hp2.tile([P, LQ], bf16, tag="dA", name="dA")
                            nc.scalar.activation(dA[:], dlt[:], AF.Exp,
                                                 scale=Acol_sb[:, dr, j, n:n + 1])
                            d1 = hp2.tile([P, LQ], bf16, tag="d1", name="d1")
                            nc.vector.tensor_tensor(d1[:], duj[:], Brep[:],
                                                    op=ALU.mult)
                            h = hp2.tile([P, LQ], bf16, tag="h", name="h")
                            stt = state[dr, n, j]
                            init = 0.0 if q == 0 else stt[:, 0:1]
                            nc.vector.tensor_tensor_scan(h[:], dA[:], d1[:],
                                                         init, op0=ALU.mult,
                                                         op1=ALU.add)
                            if q < NQ - 1:
                                nc.vector.tensor_copy(stt[:], h[:, LQ - 1:LQ])
                            hc = hp2.tile([P, LQ], bf16, tag="hc", name="hc")
                            nc.vector.tensor_tensor(hc[:], h[:], Crep[:],
                                                    op=ALU.mult)
                            for c in range(NCQ):
                                c0, c1 = c * LC, (c + 1) * LC
                                nc.tensor.matmul(m_ps[j][c][:], ident[:],
                                                 hc[:, c0:c1],
                                                 start=(n == 0), stop=False)
                    ygs = []
                    for j in range(NPAIR):
                        yg = ygp.tile([P, LQ], bf16, tag=f"yg{j}", name=f"yg{j}")
                        for c in range(NCQ):
                            c0, c1 = c * LC, (c + 1) * LC
                            nc.tensor.matmul(m_ps[j][c][:], ident[:],
                                             Du[j][:, c0:c1],
                                             start=False, stop=True)
                            nc.vector.tensor_tensor(yg[:, c0:c1], m_ps[j][c][:],
                                                    zt[j][:, c0:c1],
                                                    op=ALU.mult)
                        ygs.append(yg)

                    for ot in range(NKT):
                        for c in range(NCQ):
                            c0 = q * LQ + c * LC
                            ops_ = pso.tile([P, LC], f32, tag="outps",
                                            name="outps")
                            for j in range(NPAIR):
                                nc.tensor.matmul(
                                    ops_[:],
                                    w_out_sb[:, j, ot * P:(ot + 1) * P],
                                    ygs[j][:, c * LC:(c + 1) * LC],
                                    start=(j == 0), stop=(j == NPAIR - 1))
                            osb = op2.tile([P, LC], f32, tag="osb", name="osb")
                            nc.scalar.copy(osb[:], ops_[:])
                            nc.sync.dma_start(
                                out_dram.ap()[ot * P:(ot + 1) * P, c0:c0 + LC],
                                osb[:])
    nc.compile()
    return nc


def _prep_core_inputs(inputs, core):
    """Host-side slicing/transposition of full inputs for one core."""
    bf = ml_dtypes.bfloat16
    b = core // 4
    dtiles = [(core % 4) * NPAIR + k for k in range(NPAIR)]
    chans = np.concatenate([np.arange(dt * P, (dt + 1) * P) for dt in dtiles])

    hid = np.asarray(inputs['hidden_states'])
    w_in_full = np.asarray(inputs['in_proj_w'])
    w_out_full = np.asarray(inputs['out_proj_w'])

    per_dir = {}
    for d, sfx in enumerate(('a', 'b')):
        per_dir[d] = dict(
            cw=np.asarray(inputs[f'conv_w_{sfx}'])[chans],
            cb=np.asarray(inputs[f'conv_b_{sfx}'])[chans],
            xp=np.asarray(inputs[f'x_proj_{sfx}_w'])[:, chans],
            dtp=np.asarray(inputs[f'dt_proj_{sfx}_w'])[chans],
            dtb=np.asarray(inputs[f'dt_bias_{sfx}'])[chans],
            A=-np.exp(np.asarray(inputs[f'A_{sfx}_log'])[chans]),
            Dv=np.asarray(inputs[f'D_{sfx}'])[chans],
        )

    w_in_cols = np.empty((D, 2 * NPAIR * P), np.float32)
    for j in range(NPAIR):
        ch_j = chans[j * P:(j + 1) * P]
        w_in_cols[:, (2 * j) * P:(2 * j + 1) * P] = w_in_full[ch_j].T
        w_in_cols[:, (2 * j + 1) * P:(2 * j + 2) * P] = w_in_full[DI + ch_j].T

    out = {
        'hT': np.ascontiguousarray(hid[b].T).astype(bf),
        'w_in': np.ascontiguousarray(w_in_cols).astype(bf),
        'convw': np.ascontiguousarray(
            np.stack([per_dir[d]['cw'].reshape(NPAIR, P, KC)
                      for d in range(2)])).astype(np.float32),
        'convb': np.ascontiguousarray(
            np.stack([per_dir[d]['cb'].reshape(NPAIR, P)
                      for d in range(2)])).astype(np.float32),
        'w_xp': np.ascontiguousarray(
            np.stack([per_dir[d]['xp'].T for d in range(2)])).astype(bf),
        'w_dt': np.ascontiguousarray(
            np.stack([per_dir[d]['dtp'].reshape(NPAIR, P, DTR)
                      .transpose(0, 2, 1) for d in range(2)])).astype(bf),
        'dt_bias': np.ascontiguousarray(
            np.stack([per_dir[d]['dtb'].reshape(NPAIR, P)
                      for d in range(2)])).astype(np.float32),
        'Acol': np.ascontiguousarray(
            np.stack([per_dir[d]['A'].reshape(NPAIR, P, DS)
                      for d in range(2)])).astype(np.float32),
        'Dvec': np.ascontiguousarray(
            np.stack([per_dir[d]['Dv'].reshape(NPAIR, P)
                      for d in range(2)])).astype(np.float32),
        'w_out': np.ascontiguousarray(
            w_out_full[:, chans].T.reshape(NPAIR, P, D)).astype(bf),
        'ident': np.eye(P, dtype=np.float32).astype(bf),
    }
    return out


_module_cache = {}


def _get_module():
    if 'nc' not in _module_cache:
        _module_cache['nc'] = build_module()
    return _module_cache['nc']


def kernel(**inputs):
    nc = _get_module()
    in_maps = [_prep_core_inputs(inputs, c) for c in range(NCORES)]
    res = run_bass_kernel_spmd(nc, in_maps, list(range(NCORES)))
    out = np.zeros((B, L, D), np.float32)
    for c in range(NCORES):
        oa = np.asarray(res.results[c]['out_a'], np.float32)
        ob = np.asarray(res.results[c]['out_b'], np.float32)
        out[c // 4] += oa.T + ob[:, ::-1].T
    return out
